# revision 4
# baseline (speedup 1.0000x reference)
"""Bass/Trainium2 two-level Strassen kernel for batched GNN message passing:
    out[b, d, n] = sum_m adj[b, n, m] * x[b, d, m]
B=2, D=3072, N=8192, fp32 in/out.

Sharding: 8 cores, core c -> (b = c//4, n-quarter = c%4). Each core computes
C[3072, 2048] = X[b] @ A[b, quarter, :].T with contraction m = 8192.

Two-level Strassen (49/64 of the standard matmul work; operand combos are
prepacked on the host):
  level 1: X/Bt split 2x2 -> 7 products M_p [1536,4096]x[4096,1024]
  level 2: each M_p again 2x2 -> 7 sub-products S_{p,s} [768,2048]x[2048,512]
  C11=M1+M4-M5+M7  C12=M3+M5  C21=M2+M4  C22=M1-M2+M3+M6 (both levels)

Per core: 49 phases (outer x inner product, both in order M1,M4,M5,M7,M2,
M3,M6). Each phase: BO panel [128k, 16mc*512n] (2.1MB, double-buffered,
scalar queue), 6 d-block chains of 16 matmuls (x strips on sync queue).
Inner recombination: S1,S4,S5,S2,S3 evict PSUM->fp16 slabs; S7/S2/S3/S6
phases produce M-quadrant tiles [128,512] f32 which feed level-1
accumulation into four fp16 C-block accumulators (12.6MB SBUF; first
contribution = copy, middle = in-place add/sub, last = f32 staging + DMA
out). All DVE work (~390us) hides under the matmul stream (~1020us).
"""

import sys
from contextlib import ExitStack

import numpy as np

sys.path.insert(0, "/opt/trn_rl_repo")

B = 2
D = 3072
N = 8192
NCORES = 8
NSPLIT = 4
NC = N // NSPLIT  # 2048

P = 128
NW = 512
DH, KH, NH = D // 2, N // 2, NC // 2          # level-1 block dims
DQ, KQ, NQ = DH // 2, KH // 2, NH // 2        # level-2 block dims: 768,2048,512
NDB = DQ // P   # 6 d-blocks per product
NMC = KQ // P   # 16 mc chunks per product
NPROD = 7

ORDER = [1, 4, 5, 7, 2, 3, 6]  # phase order of products at both levels
# contributor lists in PHASE order; (i,j) C/M quadrant <- [(prod, sign)]
ALPHA = {
    (0, 0): [(1, 1), (4, 1), (5, -1), (7, 1)],
    (0, 1): [(5, 1), (3, 1)],
    (1, 0): [(4, 1), (2, 1)],
    (1, 1): [(1, 1), (2, -1), (3, 1), (6, 1)],
}
# inner quad produced at phase s: si index in ORDER -> (u,v) or None(store-only)
QUAD_AT = {3: (0, 0), 4: (1, 0), 5: (0, 1), 6: (1, 1)}
STORE_AT = {0: "s1", 1: "s4", 2: "s5", 4: "s2", 5: "s3"}

# per outer product p: list of (block(i,j), sign, kind) kind in first/mid/last
def _contrib_schedule():
    sched = {p: [] for p in ORDER}
    for blk, terms in ALPHA.items():
        for idx, (p, sg) in enumerate(terms):
            kind = "first" if idx == 0 else ("last" if idx == len(terms) - 1 else "mid")
            if kind == "first":
                assert sg == 1
            sched[p].append((blk, sg, kind))
    return sched


CONTRIB = _contrib_schedule()


def build_program():
    import concourse.mybir as mybir
    import concourse.tile as tile
    from concourse import bacc

    f32 = mybir.dt.float32
    f16 = mybir.dt.float16
    bf16 = mybir.dt.bfloat16

    nc = bacc.Bacc(None, target_bir_lowering=False, debug=False)

    # xoh[(pi*7+si)*768 + db*128 + k, mc*128 + i] = XO2[db*128+i, mc*128+k]
    xoh = nc.dram_tensor("xoh", [49 * DQ, KQ], bf16, kind="ExternalInput")
    # boh[(pi*7+si)*128 + k, mc*512 + j] = AO2[j, mc*128 + k]
    boh = nc.dram_tensor("boh", [49 * P, NMC * NW], bf16, kind="ExternalInput")
    out_ext = nc.dram_tensor("out", [D, NC], f32, kind="ExternalOutput")

    with tile.TileContext(nc) as tc, ExitStack() as ctx:
        panel_pool = ctx.enter_context(tc.tile_pool(name="panel", bufs=2))
        x_pool = ctx.enter_context(tc.tile_pool(name="xp", bufs=6))
        s_pool = ctx.enter_context(tc.tile_pool(name="sp", bufs=5))
        c_pool = ctx.enter_context(tc.tile_pool(name="cp", bufs=4))
        o_pool = ctx.enter_context(tc.tile_pool(name="op", bufs=8))
        acc_psum = ctx.enter_context(tc.tile_pool(name="accp", bufs=6, space="PSUM"))

        # four persistent fp16 C-block accumulators [128, 12*1024]
        cacc = {
            blk: c_pool.tile([P, 12 * 1024], f16, tag="c", name=f"c{blk[0]}{blk[1]}")
            for blk in ALPHA
        }

        def cslice(blk, u, v, db):
            base = (u * NDB + db) * 1024 + v * NW
            return cacc[blk][:, base : base + NW]

        def load_x(ph, db, piece_mcs, name="xs"):
            xs = x_pool.tile([P, KQ], bf16, tag="xs", name=name)
            lo = 0
            for n in piece_mcs:
                nc.sync.dma_start(
                    out=xs[:, lo * P : (lo + n) * P],
                    in_=xoh[
                        ph * DQ + db * P : ph * DQ + (db + 1) * P,
                        lo * P : (lo + n) * P,
                    ],
                )
                lo += n
            return xs

        def load_panel(ph, piece_mcs):
            panel = panel_pool.tile([P, NMC * NW], bf16, tag="panel")
            lo = 0
            for n in piece_mcs:
                nc.scalar.dma_start(
                    out=panel[:, lo * NW : (lo + n) * NW],
                    in_=boh[ph * P : (ph + 1) * P, lo * NW : (lo + n) * NW],
                )
                lo += n
            return panel

        def mm_chain(acc, xs, panel, mcs):
            for mc in mcs:
                nc.tensor.matmul(
                    acc[:],
                    xs[:, mc * P : (mc + 1) * P],
                    panel[:, mc * NW : (mc + 1) * NW],
                    start=(mc == 0),
                    stop=(mc == NMC - 1),
                )

        stiles = {}

        def scol(name, db):
            return stiles[name][:, db * NW : (db + 1) * NW]

        def outer_emit(p, u, v, db, mq, last_kernel=False):
            """mq: [128,512] f32 AP holding M_p quadrant (u,v) d-block db."""
            for blk, sg, kind in CONTRIB[p]:
                i, j = blk
                tgt = cslice(blk, u, v, db)
                if kind == "first":
                    nc.vector.tensor_copy(out=tgt, in_=mq)
                elif kind == "mid":
                    if sg > 0:
                        nc.vector.tensor_add(tgt, tgt, mq)
                    else:
                        nc.vector.tensor_sub(tgt, tgt, mq)
                else:  # last: f32 staging + DMA out (scalar queue, off the
                    # x-strip sync queue)
                    cout = o_pool.tile([P, NW], f32, tag="t", name="cout")
                    if sg > 0:
                        nc.vector.tensor_add(cout[:], tgt, mq)
                    else:
                        nc.vector.tensor_sub(cout[:], tgt, mq)
                    row0 = i * DH + u * DQ + db * P
                    col0 = j * NH + v * NW
                    split = 4 if last_kernel else 1
                    w = NW // split
                    for t in range(split):
                        nc.scalar.dma_start(
                            out=out_ext[row0 : row0 + P, col0 + t * w : col0 + (t + 1) * w],
                            in_=cout[:, t * w : (t + 1) * w],
                        )

        def recombine(p, si_idx, db, acc, last_kernel=False):
            if si_idx in STORE_AT:
                name = STORE_AT[si_idx]
                if db == 0:
                    stiles[name] = s_pool.tile([P, NDB * NW], f16, tag="s", name=name)
                nc.vector.tensor_copy(out=scol(name, db), in_=acc[:])
            if si_idx == 3:  # S7 -> quad (0,0) = S1 + S4 - S5 + S7
                ta = o_pool.tile([P, NW], f32, tag="t", name="ta")
                tb = o_pool.tile([P, NW], f32, tag="t", name="tb")
                mq = o_pool.tile([P, NW], f32, tag="t", name="mq")
                nc.vector.tensor_add(ta[:], acc[:], scol("s1", db))
                nc.vector.tensor_add(tb[:], ta[:], scol("s4", db))
                nc.vector.tensor_sub(mq[:], tb[:], scol("s5", db))
                outer_emit(p, 0, 0, db, mq[:], last_kernel)
            elif si_idx == 4:  # S2 -> quad (1,0) = S2 + S4
                mq = o_pool.tile([P, NW], f32, tag="t", name="mq")
                nc.vector.tensor_add(mq[:], acc[:], scol("s4", db))
                outer_emit(p, 1, 0, db, mq[:], last_kernel)
            elif si_idx == 5:  # S3 -> quad (0,1) = S3 + S5
                mq = o_pool.tile([P, NW], f32, tag="t", name="mq")
                nc.vector.tensor_add(mq[:], acc[:], scol("s5", db))
                outer_emit(p, 0, 1, db, mq[:], last_kernel)
            elif si_idx == 6:  # S6 -> quad (1,1) = S1 - S2 + S3 + S6
                ta = o_pool.tile([P, NW], f32, tag="t", name="ta")
                tb = o_pool.tile([P, NW], f32, tag="t", name="tb")
                mq = o_pool.tile([P, NW], f32, tag="t", name="mq")
                nc.vector.tensor_add(ta[:], acc[:], scol("s1", db))
                nc.vector.tensor_sub(tb[:], ta[:], scol("s2", db))
                nc.vector.tensor_add(mq[:], tb[:], scol("s3", db))
                outer_emit(p, 1, 1, db, mq[:], last_kernel)

        for pi_idx in range(NPROD):
            p = ORDER[pi_idx]
            for si_idx in range(NPROD):
                ph = pi_idx * NPROD + si_idx
                is_last_phase = ph == 48
                if ph == 0:
                    # startup: interleave first NI chains with graduated panel
                    # pieces (panel ~2.1MB streams slower than one chain's
                    # matmuls; 5 chains x 3.5us of work cover the stream)
                    piece_mcs = [1, 1, 2, 4, 8]
                    panel = load_panel(ph, piece_mcs)
                    NI = 5
                    xtiles = [
                        x_pool.tile([P, KQ], bf16, tag="xs", name=f"x0{i}")
                        for i in range(NI)
                    ]
                    lo = 0
                    for n in [1, 2, 5, 8]:
                        for db in range(NI):
                            nc.sync.dma_start(
                                out=xtiles[db][:, lo * P : (lo + n) * P],
                                in_=xoh[db * P : (db + 1) * P, lo * P : (lo + n) * P],
                            )
                        lo += n
                    accs = [
                        acc_psum.tile([P, NW], f32, tag="acc", name=f"acc{i}")
                        for i in range(NI)
                    ]
                    lo = 0
                    for n in piece_mcs:
                        for db in range(NI):
                            mm_chain(accs[db], xtiles[db], panel, range(lo, lo + n))
                        lo += n
                    for db in range(NI):
                        recombine(p, si_idx, db, accs[db])
                    rest = range(NI, NDB)
                else:
                    panel = load_panel(ph, [8, 8])
                    rest = range(NDB)

                for db in rest:
                    xs = load_x(ph, db, [8, 8])
                    acc = acc_psum.tile([P, NW], f32, tag="acc")
                    mm_chain(acc, xs, panel, range(NMC))
                    recombine(
                        p, si_idx, db, acc,
                        last_kernel=(is_last_phase and db == NDB - 1),
                    )

    nc.compile()
    return nc


_NC_CACHE = {}


def _get_program():
    if "nc" not in _NC_CACHE:
        _NC_CACHE["nc"] = build_program()
    return _NC_CACHE["nc"]


def _split4(M, r, c):
    return M[:r, :c], M[:r, c:], M[r:, :c], M[r:, c:]


_XCOMBO = {
    1: lambda a, b, c, d: a + d,
    2: lambda a, b, c, d: c + d,
    3: lambda a, b, c, d: a,
    4: lambda a, b, c, d: d,
    5: lambda a, b, c, d: a + b,
    6: lambda a, b, c, d: c - a,
    7: lambda a, b, c, d: b - d,
}
# B-side combos expressed on AO blocks (AO = BO^T): AO2_s over
# (Z11,Z12,Z21,Z22) in AO form: Z11=AO[:n2,:k2] Z12=AO[n2:,:k2]
# Z21=AO[:n2,k2:] Z22=AO[n2:,k2:]
_BCOMBO = {
    1: lambda z11, z12, z21, z22: z11 + z22,
    2: lambda z11, z12, z21, z22: z11,
    3: lambda z11, z12, z21, z22: z12 - z22,
    4: lambda z11, z12, z21, z22: z21 - z11,
    5: lambda z11, z12, z21, z22: z22,
    6: lambda z11, z12, z21, z22: z11 + z12,
    7: lambda z11, z12, z21, z22: z21 + z22,
}


def prepare_in_maps(x: np.ndarray, adj: np.ndarray) -> list:
    """Host-side prepack: two-level Strassen combos, transpose-tiled bf16."""
    import ml_dtypes

    bf16 = ml_dtypes.bfloat16

    def pack_xo2(XO):  # [768, 2048] f32 -> [768, 2048] bf16 tile layout
        xt = np.ascontiguousarray(XO.T).astype(bf16)  # [2048, 768]
        return xt.reshape(NMC, P, NDB, P).transpose(2, 1, 0, 3).reshape(DQ, KQ)

    def pack_ao2(AO):  # [512, 2048] f32 -> [128, 8192] bf16
        a = AO.astype(bf16)
        return a.reshape(NW, NMC, P).transpose(2, 1, 0).reshape(P, NMC * NW)

    xoh_by_b = []
    for b in range(B):
        X = x[b]
        Xb = _split4(X, DH, KH)
        xoh = np.empty((49 * DQ, KQ), dtype=bf16)
        for pi_idx, p in enumerate(ORDER):
            XO = _XCOMBO[p](*Xb)  # [1536, 4096]
            Yb = _split4(XO, DQ, KQ)
            for si_idx, s in enumerate(ORDER):
                XO2 = _XCOMBO[s](*Yb)  # [768, 2048]
                ph = pi_idx * NPROD + si_idx
                xoh[ph * DQ : (ph + 1) * DQ] = pack_xo2(XO2)
        xoh_by_b.append(xoh)

    in_maps = []
    for c in range(NCORES):
        b, q = divmod(c, NSPLIT)
        A = adj[b, q * NC : (q + 1) * NC, :]  # [2048, 8192]
        # outer AO blocks: AO_p = B-combos of A quadrants, AO form [1024,4096]
        A1l = A[:NH, :KH]
        A1r = A[:NH, KH:]
        A2l = A[NH:, :KH]
        A2r = A[NH:, KH:]
        zb_outer = (A1l, A2l, A1r, A2r)  # z11, z12, z21, z22 in AO form
        boh = np.empty((49 * P, NMC * NW), dtype=bf16)
        for pi_idx, p in enumerate(ORDER):
            AO = _BCOMBO[p](*zb_outer)  # [1024, 4096]
            z_in = (AO[:NQ, :KQ], AO[NQ:, :KQ], AO[:NQ, KQ:], AO[NQ:, KQ:])
            for si_idx, s in enumerate(ORDER):
                AO2 = _BCOMBO[s](*z_in)  # [512, 2048]
                ph = pi_idx * NPROD + si_idx
                boh[ph * P : (ph + 1) * P] = pack_ao2(AO2)
        in_maps.append({"xoh": xoh_by_b[b], "boh": np.ascontiguousarray(boh)})
    return in_maps


def kernel(x: np.ndarray, adj: np.ndarray) -> np.ndarray:
    """Full inputs in, full output out. x [B,D,N] f32, adj [B,N,N] f32."""
    from concourse.bass_utils import run_bass_kernel_spmd

    assert x.shape == (B, D, N) and adj.shape == (B, N, N)
    nc = _get_program()
    in_maps = prepare_in_maps(np.asarray(x), np.asarray(adj))

    res = run_bass_kernel_spmd(nc, in_maps, core_ids=list(range(NCORES)))
    out = np.empty((B, D, N), dtype=np.float32)
    for c in range(NCORES):
        b, q = divmod(c, NSPLIT)
        out[b, :, q * NC : (q + 1) * NC] = res.results[c]["out"]
    return out


# revision 5
# speedup vs baseline: 1.0041x; 1.0041x over previous
"""Bass/Trainium2 two-level Strassen kernel for batched GNN message passing:
    out[b, d, n] = sum_m adj[b, n, m] * x[b, d, m]
B=2, D=3072, N=8192, fp32 in/out.

Sharding: 8 cores, core c -> (b = c//4, n-quarter = c%4). Each core computes
C[3072, 2048] = X[b] @ A[b, quarter, :].T with contraction m = 8192.

Two-level Strassen (49/64 of the standard matmul work; operand combos are
prepacked on the host):
  level 1: X/Bt split 2x2 -> 7 products M_p [1536,4096]x[4096,1024]
  level 2: each M_p again 2x2 -> 7 sub-products S_{p,s} [768,2048]x[2048,512]
  C11=M1+M4-M5+M7  C12=M3+M5  C21=M2+M4  C22=M1-M2+M3+M6 (both levels)

Per core: 49 phases (outer x inner product, both in order M1,M4,M5,M7,M2,
M3,M6). Each phase: BO panel [128k, 16mc*512n] (2.1MB, double-buffered,
scalar queue), 6 d-block chains of 16 matmuls (x strips on sync queue).
Inner recombination: S1,S4,S5,S2,S3 evict PSUM->fp16 slabs; S7/S2/S3/S6
phases produce M-quadrant tiles [128,512] f32 which feed level-1
accumulation into four fp16 C-block accumulators (12.6MB SBUF; first
contribution = copy, middle = in-place add/sub, last = f32 staging + DMA
out). All DVE work (~390us) hides under the matmul stream (~1020us).
"""

import sys
from contextlib import ExitStack

import numpy as np

sys.path.insert(0, "/opt/trn_rl_repo")

B = 2
D = 3072
N = 8192
NCORES = 8
NSPLIT = 4
NC = N // NSPLIT  # 2048

P = 128
NW = 512
DH, KH, NH = D // 2, N // 2, NC // 2          # level-1 block dims
DQ, KQ, NQ = DH // 2, KH // 2, NH // 2        # level-2 block dims: 768,2048,512
NDB = DQ // P   # 6 d-blocks per product
NMC = KQ // P   # 16 mc chunks per product
NPROD = 7

ORDER = [1, 4, 5, 7, 2, 3, 6]  # phase order of products at both levels
# contributor lists in PHASE order; (i,j) C/M quadrant <- [(prod, sign)]
ALPHA = {
    (0, 0): [(1, 1), (4, 1), (5, -1), (7, 1)],
    (0, 1): [(5, 1), (3, 1)],
    (1, 0): [(4, 1), (2, 1)],
    (1, 1): [(1, 1), (2, -1), (3, 1), (6, 1)],
}
# inner quad produced at phase s: si index in ORDER -> (u,v) or None(store-only)
QUAD_AT = {3: (0, 0), 4: (1, 0), 5: (0, 1), 6: (1, 1)}
STORE_AT = {0: "s1", 1: "s4", 2: "s5", 4: "s2", 5: "s3"}

# per outer product p: list of (block(i,j), sign, kind) kind in first/mid/last
def _contrib_schedule():
    sched = {p: [] for p in ORDER}
    for blk, terms in ALPHA.items():
        for idx, (p, sg) in enumerate(terms):
            kind = "first" if idx == 0 else ("last" if idx == len(terms) - 1 else "mid")
            if kind == "first":
                assert sg == 1
            sched[p].append((blk, sg, kind))
    return sched


CONTRIB = _contrib_schedule()


def build_program():
    import concourse.mybir as mybir
    import concourse.tile as tile
    from concourse import bacc

    f32 = mybir.dt.float32
    f16 = mybir.dt.float16
    bf16 = mybir.dt.bfloat16

    nc = bacc.Bacc(None, target_bir_lowering=False, debug=False)

    # xoh[(pi*7+si)*768 + db*128 + k, mc*128 + i] = XO2[db*128+i, mc*128+k]
    xoh = nc.dram_tensor("xoh", [49 * DQ, KQ], bf16, kind="ExternalInput")
    # boh[(pi*7+si)*128 + k, mc*512 + j] = AO2[j, mc*128 + k]
    boh = nc.dram_tensor("boh", [49 * P, NMC * NW], bf16, kind="ExternalInput")
    out_ext = nc.dram_tensor("out", [D, NC], f32, kind="ExternalOutput")

    with tile.TileContext(nc) as tc, ExitStack() as ctx:
        panel_pool = ctx.enter_context(tc.tile_pool(name="panel", bufs=2))
        x_pool = ctx.enter_context(tc.tile_pool(name="xp", bufs=6))
        s_pool = ctx.enter_context(tc.tile_pool(name="sp", bufs=5))
        c_pool = ctx.enter_context(tc.tile_pool(name="cp", bufs=4))
        o_pool = ctx.enter_context(tc.tile_pool(name="op", bufs=8))
        acc_psum = ctx.enter_context(tc.tile_pool(name="accp", bufs=6, space="PSUM"))

        # four persistent fp16 C-block accumulators [128, 12*1024]
        cacc = {
            blk: c_pool.tile([P, 12 * 1024], f16, tag="c", name=f"c{blk[0]}{blk[1]}")
            for blk in ALPHA
        }

        def cslice(blk, u, v, db):
            base = (u * NDB + db) * 1024 + v * NW
            return cacc[blk][:, base : base + NW]

        def load_x(ph, db, piece_mcs, name="xs"):
            xs = x_pool.tile([P, KQ], bf16, tag="xs", name=name)
            lo = 0
            for n in piece_mcs:
                nc.sync.dma_start(
                    out=xs[:, lo * P : (lo + n) * P],
                    in_=xoh[
                        ph * DQ + db * P : ph * DQ + (db + 1) * P,
                        lo * P : (lo + n) * P,
                    ],
                )
                lo += n
            return xs

        def load_panel(ph, piece_mcs):
            panel = panel_pool.tile([P, NMC * NW], bf16, tag="panel")
            lo = 0
            for n in piece_mcs:
                nc.scalar.dma_start(
                    out=panel[:, lo * NW : (lo + n) * NW],
                    in_=boh[ph * P : (ph + 1) * P, lo * NW : (lo + n) * NW],
                )
                lo += n
            return panel

        def mm_chain(acc, xs, panel, mcs):
            for mc in mcs:
                nc.tensor.matmul(
                    acc[:],
                    xs[:, mc * P : (mc + 1) * P],
                    panel[:, mc * NW : (mc + 1) * NW],
                    start=(mc == 0),
                    stop=(mc == NMC - 1),
                )

        stiles = {}

        def scol(name, db):
            return stiles[name][:, db * NW : (db + 1) * NW]

        def outer_emit(p, u, v, db, mq, last_kernel=False):
            """mq: [128,512] f32 AP holding M_p quadrant (u,v) d-block db."""
            for blk, sg, kind in CONTRIB[p]:
                i, j = blk
                tgt = cslice(blk, u, v, db)
                if kind == "first":
                    nc.vector.tensor_copy(out=tgt, in_=mq)
                elif kind == "mid":
                    if sg > 0:
                        nc.vector.tensor_add(tgt, tgt, mq)
                    else:
                        nc.vector.tensor_sub(tgt, tgt, mq)
                else:  # last: f32 staging + DMA out
                    cout = o_pool.tile([P, NW], f32, tag="t", name="cout")
                    if sg > 0:
                        nc.vector.tensor_add(cout[:], tgt, mq)
                    else:
                        nc.vector.tensor_sub(cout[:], tgt, mq)
                    row0 = i * DH + u * DQ + db * P
                    col0 = j * NH + v * NW
                    split = 2 if last_kernel else 1
                    w = NW // split
                    for t in range(split):
                        nc.sync.dma_start(
                            out=out_ext[row0 : row0 + P, col0 + t * w : col0 + (t + 1) * w],
                            in_=cout[:, t * w : (t + 1) * w],
                        )

        def recombine(p, si_idx, db, acc, last_kernel=False):
            if si_idx in STORE_AT:
                name = STORE_AT[si_idx]
                if db == 0:
                    stiles[name] = s_pool.tile([P, NDB * NW], f16, tag="s", name=name)
                nc.vector.tensor_copy(out=scol(name, db), in_=acc[:])
            if si_idx == 3:  # S7 -> quad (0,0) = S1 + S4 - S5 + S7
                ta = o_pool.tile([P, NW], f32, tag="t", name="ta")
                tb = o_pool.tile([P, NW], f32, tag="t", name="tb")
                mq = o_pool.tile([P, NW], f32, tag="t", name="mq")
                nc.vector.tensor_add(ta[:], acc[:], scol("s1", db))
                nc.vector.tensor_add(tb[:], ta[:], scol("s4", db))
                nc.vector.tensor_sub(mq[:], tb[:], scol("s5", db))
                outer_emit(p, 0, 0, db, mq[:], last_kernel)
            elif si_idx == 4:  # S2 -> quad (1,0) = S2 + S4
                mq = o_pool.tile([P, NW], f32, tag="t", name="mq")
                nc.vector.tensor_add(mq[:], acc[:], scol("s4", db))
                outer_emit(p, 1, 0, db, mq[:], last_kernel)
            elif si_idx == 5:  # S3 -> quad (0,1) = S3 + S5
                mq = o_pool.tile([P, NW], f32, tag="t", name="mq")
                nc.vector.tensor_add(mq[:], acc[:], scol("s5", db))
                outer_emit(p, 0, 1, db, mq[:], last_kernel)
            elif si_idx == 6:  # S6 -> quad (1,1) = S1 - S2 + S3 + S6
                ta = o_pool.tile([P, NW], f32, tag="t", name="ta")
                tb = o_pool.tile([P, NW], f32, tag="t", name="tb")
                mq = o_pool.tile([P, NW], f32, tag="t", name="mq")
                nc.vector.tensor_add(ta[:], acc[:], scol("s1", db))
                nc.vector.tensor_sub(tb[:], ta[:], scol("s2", db))
                nc.vector.tensor_add(mq[:], tb[:], scol("s3", db))
                outer_emit(p, 1, 1, db, mq[:], last_kernel)

        for pi_idx in range(NPROD):
            p = ORDER[pi_idx]
            for si_idx in range(NPROD):
                ph = pi_idx * NPROD + si_idx
                is_last_phase = ph == 48
                if ph == 0:
                    # startup: interleave first NI chains with graduated panel
                    piece_mcs = [1, 1, 2, 4, 8]
                    panel = load_panel(ph, piece_mcs)
                    NI = 3
                    xtiles = [
                        x_pool.tile([P, KQ], bf16, tag="xs", name=f"x0{i}")
                        for i in range(NI)
                    ]
                    lo = 0
                    for n in [1, 2, 5, 8]:
                        for db in range(NI):
                            nc.sync.dma_start(
                                out=xtiles[db][:, lo * P : (lo + n) * P],
                                in_=xoh[db * P : (db + 1) * P, lo * P : (lo + n) * P],
                            )
                        lo += n
                    accs = [
                        acc_psum.tile([P, NW], f32, tag="acc", name=f"acc{i}")
                        for i in range(NI)
                    ]
                    lo = 0
                    for n in piece_mcs:
                        for db in range(NI):
                            mm_chain(accs[db], xtiles[db], panel, range(lo, lo + n))
                        lo += n
                    for db in range(NI):
                        recombine(p, si_idx, db, accs[db])
                    rest = range(NI, NDB)
                else:
                    panel = load_panel(ph, [8, 8])
                    rest = range(NDB)

                for db in rest:
                    xs = load_x(ph, db, [8, 8])
                    acc = acc_psum.tile([P, NW], f32, tag="acc")
                    mm_chain(acc, xs, panel, range(NMC))
                    recombine(
                        p, si_idx, db, acc,
                        last_kernel=(is_last_phase and db == NDB - 1),
                    )

    nc.compile()
    return nc


_NC_CACHE = {}


def _get_program():
    if "nc" not in _NC_CACHE:
        _NC_CACHE["nc"] = build_program()
    return _NC_CACHE["nc"]


def _split4(M, r, c):
    return M[:r, :c], M[:r, c:], M[r:, :c], M[r:, c:]


_XCOMBO = {
    1: lambda a, b, c, d: a + d,
    2: lambda a, b, c, d: c + d,
    3: lambda a, b, c, d: a,
    4: lambda a, b, c, d: d,
    5: lambda a, b, c, d: a + b,
    6: lambda a, b, c, d: c - a,
    7: lambda a, b, c, d: b - d,
}
# B-side combos expressed on AO blocks (AO = BO^T): AO2_s over
# (Z11,Z12,Z21,Z22) in AO form: Z11=AO[:n2,:k2] Z12=AO[n2:,:k2]
# Z21=AO[:n2,k2:] Z22=AO[n2:,k2:]
_BCOMBO = {
    1: lambda z11, z12, z21, z22: z11 + z22,
    2: lambda z11, z12, z21, z22: z11,
    3: lambda z11, z12, z21, z22: z12 - z22,
    4: lambda z11, z12, z21, z22: z21 - z11,
    5: lambda z11, z12, z21, z22: z22,
    6: lambda z11, z12, z21, z22: z11 + z12,
    7: lambda z11, z12, z21, z22: z21 + z22,
}


def prepare_in_maps(x: np.ndarray, adj: np.ndarray) -> list:
    """Host-side prepack: two-level Strassen combos, transpose-tiled bf16."""
    import ml_dtypes

    bf16 = ml_dtypes.bfloat16

    def pack_xo2(XO):  # [768, 2048] f32 -> [768, 2048] bf16 tile layout
        xt = np.ascontiguousarray(XO.T).astype(bf16)  # [2048, 768]
        return xt.reshape(NMC, P, NDB, P).transpose(2, 1, 0, 3).reshape(DQ, KQ)

    def pack_ao2(AO):  # [512, 2048] f32 -> [128, 8192] bf16
        a = AO.astype(bf16)
        return a.reshape(NW, NMC, P).transpose(2, 1, 0).reshape(P, NMC * NW)

    xoh_by_b = []
    for b in range(B):
        X = x[b]
        Xb = _split4(X, DH, KH)
        xoh = np.empty((49 * DQ, KQ), dtype=bf16)
        for pi_idx, p in enumerate(ORDER):
            XO = _XCOMBO[p](*Xb)  # [1536, 4096]
            Yb = _split4(XO, DQ, KQ)
            for si_idx, s in enumerate(ORDER):
                XO2 = _XCOMBO[s](*Yb)  # [768, 2048]
                ph = pi_idx * NPROD + si_idx
                xoh[ph * DQ : (ph + 1) * DQ] = pack_xo2(XO2)
        xoh_by_b.append(xoh)

    in_maps = []
    for c in range(NCORES):
        b, q = divmod(c, NSPLIT)
        A = adj[b, q * NC : (q + 1) * NC, :]  # [2048, 8192]
        # outer AO blocks: AO_p = B-combos of A quadrants, AO form [1024,4096]
        A1l = A[:NH, :KH]
        A1r = A[:NH, KH:]
        A2l = A[NH:, :KH]
        A2r = A[NH:, KH:]
        zb_outer = (A1l, A2l, A1r, A2r)  # z11, z12, z21, z22 in AO form
        boh = np.empty((49 * P, NMC * NW), dtype=bf16)
        for pi_idx, p in enumerate(ORDER):
            AO = _BCOMBO[p](*zb_outer)  # [1024, 4096]
            z_in = (AO[:NQ, :KQ], AO[NQ:, :KQ], AO[:NQ, KQ:], AO[NQ:, KQ:])
            for si_idx, s in enumerate(ORDER):
                AO2 = _BCOMBO[s](*z_in)  # [512, 2048]
                ph = pi_idx * NPROD + si_idx
                boh[ph * P : (ph + 1) * P] = pack_ao2(AO2)
        in_maps.append({"xoh": xoh_by_b[b], "boh": np.ascontiguousarray(boh)})
    return in_maps


def kernel(x: np.ndarray, adj: np.ndarray) -> np.ndarray:
    """Full inputs in, full output out. x [B,D,N] f32, adj [B,N,N] f32."""
    from concourse.bass_utils import run_bass_kernel_spmd

    assert x.shape == (B, D, N) and adj.shape == (B, N, N)
    nc = _get_program()
    in_maps = prepare_in_maps(np.asarray(x), np.asarray(adj))

    res = run_bass_kernel_spmd(nc, in_maps, core_ids=list(range(NCORES)))
    out = np.empty((B, D, N), dtype=np.float32)
    for c in range(NCORES):
        b, q = divmod(c, NSPLIT)
        out[b, :, q * NC : (q + 1) * NC] = res.results[c]["out"]
    return out
